# revision 9
# baseline (speedup 1.0000x reference)
"""Bass/Trainium2 kernel for nn_GraphTextModel (loss_fn).

Strategy (data-parallel over m=65536 rows, 8 cores x 8192 rows):
  Host:   pre-transpose x / gathered-embedding slices so the PE contracts
          along partitions; gather emb_table rows; spread LSTM gate weights
          to 32-aligned partition groups [i@0:20 | o@32:52 | g@64:84]
          (f-gate unused since c0=0).
  Device: aT = W1T.T @ xT, relu+bias via ACT      -> sh[0:64]
          gatesT = WgT.T @ x0T (+bias in ACT), sigmoid/tanh -> h1sT
          ftT = Wf @ h1sT (+bf in ACT)            -> output [64, 8192]
          one PE-transpose per 128-row chunk of the stacked [a|h] tile,
          one Gram matmul into a PSUM accumulator:
             sG = [a|h|1]^T @ [a|h|1]  (85x85, PSUM-resident)
  Host:   combine the 8 partial Gram stats, form the 20x20 sigma matrices,
          inv-sqrtm (eigh) + SVD in float64, assemble (loss, var_g, var_f, ft).

The big matmul movers use float32r (relaxed-precision fp32 multiply at 1
cycle/row vs 4 for full fp32); the Gram/transpose/ft path stays full fp32.
"""

import os
import sys

import numpy as np

try:  # pragma: no cover - environment plumbing
    import concourse  # noqa: F401
except ImportError:
    sys.path.insert(0, "/opt/trn_rl_repo")

M = 65536
PSD = 1024
H1 = 64
D = 20
EMB = 300
YD = 64
NCORES = 8
MC = M // NCORES  # 8192 rows per core
BB = 1024  # columns per DMA block (4 KiB bursts)
NBB = MC // BB  # 8
B = 512  # columns per compute half-block (PSUM bank limit for fp32)
KX = PSD // 128  # 8 contraction chunks for node_repr
GW = 85  # gram width: 64 (a) + 20 (h) + 1 (ones)
REG1 = 0.001
REG2 = 0.001

F32R = os.environ.get("KERNEL_F32R", "1") == "1"

_PROG = {}
LAST_EXEC_NS = None
LAST_RESULTS = None


def _build():
    key = ("nc", F32R)
    if key in _PROG:
        return _PROG[key]
    from contextlib import ExitStack

    import concourse.tile as tile
    from concourse import bacc, mybir
    from concourse.masks import make_identity

    f32 = mybir.dt.float32
    fmv = mybir.dt.float32r if F32R else f32  # dtype of the big movers
    AF = mybir.ActivationFunctionType

    nc = bacc.Bacc(
        "TRN2", target_bir_lowering=False, debug=False, num_devices=NCORES
    )

    xT = nc.dram_tensor("xT", [PSD, MC], fmv, kind="ExternalInput").ap()
    x0T = nc.dram_tensor("x0T", [EMB, MC], fmv, kind="ExternalInput").ap()
    w1T = nc.dram_tensor("w1T", [PSD, H1], fmv, kind="ExternalInput").ap()
    wgT = nc.dram_tensor("wgT", [EMB, 96], fmv, kind="ExternalInput").ap()
    b1c = nc.dram_tensor("b1c", [H1, 1], f32, kind="ExternalInput").ap()
    bg96 = nc.dram_tensor("bg96", [96, 1], f32, kind="ExternalInput").ap()
    wfT = nc.dram_tensor("wfT", [D, YD], fmv, kind="ExternalInput").ap()
    bf2 = nc.dram_tensor("bf2", [YD, 1], f32, kind="ExternalInput").ap()

    ftT = nc.dram_tensor("ftT", [YD, MC], f32, kind="ExternalOutput").ap()
    sG = nc.dram_tensor("sG", [GW, GW], f32, kind="ExternalOutput").ap()

    with tile.TileContext(nc) as tc, ExitStack() as ctx:
        wpool = ctx.enter_context(tc.tile_pool(name="w", bufs=1))
        xpool = ctx.enter_context(tc.tile_pool(name="xp", bufs=2))
        x0pool = ctx.enter_context(tc.tile_pool(name="x0p", bufs=3))
        spool = ctx.enter_context(tc.tile_pool(name="sp", bufs=2))
        fpool = ctx.enter_context(tc.tile_pool(name="fp", bufs=2))
        ahpool = ctx.enter_context(tc.tile_pool(name="ahp", bufs=4))
        outp = ctx.enter_context(tc.tile_pool(name="outp", bufs=2))

        ps_a = ctx.enter_context(tc.tile_pool(name="ps_a", bufs=2, space="PSUM"))
        ps_g = ctx.enter_context(tc.tile_pool(name="ps_g", bufs=2, space="PSUM"))
        ps_f = ctx.enter_context(tc.tile_pool(name="ps_f", bufs=1, space="PSUM"))
        ps_t = ctx.enter_context(tc.tile_pool(name="ps_t", bufs=2, space="PSUM"))
        ps_acc = ctx.enter_context(tc.tile_pool(name="ps_acc", bufs=1, space="PSUM"))

        ident = wpool.tile([128, 128], f32)
        make_identity(nc, ident[:, :])

        w1 = wpool.tile([128, KX * H1], fmv)
        for c in range(KX):
            nc.gpsimd.dma_start(
                w1[:, c * H1 : (c + 1) * H1], w1T[c * 128 : (c + 1) * 128, :]
            )
        wg = wpool.tile([128, 288], fmv)
        nc.gpsimd.dma_start(wg[:, 0:96], wgT[0:128, :])
        nc.gpsimd.dma_start(wg[:, 96:192], wgT[128:256, :])
        nc.gpsimd.dma_start(wg[0:44, 192:288], wgT[256:300, :])

        b1t = wpool.tile([H1, 1], f32)
        nc.gpsimd.dma_start(b1t[:, :], b1c[:, :])
        bgt = wpool.tile([96, 1], f32)
        nc.gpsimd.dma_start(bgt[:, :], bg96[:, :])
        wf = wpool.tile([D, YD], fmv)
        nc.gpsimd.dma_start(wf[:, :], wfT[:, :])
        bft = wpool.tile([YD, 1], f32)
        nc.gpsimd.dma_start(bft[:, :], bf2[:, :])

        accG = ps_acc.tile([GW, GW], f32)

        # variable-width x blocks: 1024 for pipeline ramp-up, then 2048 so
        # DMA packets reach 8 KiB/partition-run; x0/gates stay 1024-granular
        XBLOCKS = [(0, 1024), (1024, 1024), (2048, 2048), (4096, 2048), (6144, 2048)]
        NH = MC // B  # 16 compute halves of 512 cols
        x0t = None
        x0_pos = -1
        t = 0
        for xoff, xw in XBLOCKS:
            xt = xpool.tile([128, KX * xw], fmv, tag="xt")
            for c in range(KX):
                nc.sync.dma_start(
                    xt[:, c * xw : (c + 1) * xw],
                    xT[c * 128 : (c + 1) * 128, xoff : xoff + xw],
                )
            for hh in range(xw // B):
                pos = xoff + hh * B
                if pos // 1024 != x0_pos:
                    x0_pos = pos // 1024
                    x0c = slice(x0_pos * 1024, (x0_pos + 1) * 1024)
                    x0t = x0pool.tile([128, 3 * 1024], fmv, tag="x0t")
                    nc.gpsimd.dma_start(x0t[:, 0:1024], x0T[0:128, x0c])
                    nc.gpsimd.dma_start(x0t[:, 1024:2048], x0T[128:256, x0c])
                    nc.gpsimd.dma_start(x0t[0:44, 2048:3072], x0T[256:300, x0c])
                    f_s = fpool.tile([YD, 1024], f32, tag="f_s")
                xo = pos - x0_pos * 1024

                a_ps = ps_a.tile([H1, B], f32)
                for c in range(KX):
                    nc.tensor.matmul(
                        a_ps[:, :],
                        w1[:, c * H1 : (c + 1) * H1],
                        xt[:, c * xw + hh * B : c * xw + hh * B + B],
                        start=(c == 0),
                        stop=(c == KX - 1),
                    )

                g_ps = ps_g.tile([96, B], f32)
                nc.tensor.matmul(
                    g_ps[:, :],
                    wg[:, 0:96],
                    x0t[:, xo : xo + B],
                    start=True,
                    stop=False,
                )
                nc.tensor.matmul(
                    g_ps[:, :],
                    wg[:, 96:192],
                    x0t[:, 1024 + xo : 1024 + xo + B],
                    start=False,
                    stop=False,
                )
                nc.tensor.matmul(
                    g_ps[:, :],
                    wg[0:44, 192:288],
                    x0t[0:44, 2048 + xo : 2048 + xo + B],
                    start=False,
                    stop=True,
                )

                # stacked [a | h] tile: relu(a)+b1 -> rows 0:64, h1s -> 64:84
                sh = spool.tile([H1 + D, B], f32, tag="sh")
                nc.scalar.activation(
                    sh[0:H1, :], a_ps[:, :], AF.Relu, bias=b1t[:, :]
                )

                # one sigmoid over [i@0:20 | junk | o@32:52]
                sio = spool.tile([52, B], f32, tag="sio")
                nc.scalar.activation(
                    sio[:, :], g_ps[0:52, :], AF.Sigmoid, bias=bgt[0:52, :]
                )
                tg = spool.tile([D, B], f32, tag="tg")
                nc.scalar.activation(
                    tg[:, :], g_ps[64:84, :], AF.Tanh, bias=bgt[64:84, :]
                )
                c1 = spool.tile([D, B], f32, tag="c1")
                nc.vector.tensor_mul(c1[:, :], sio[0:20, :], tg[:, :])
                # tanh(c1) written at base 32 so the h-mul has equal-base ins
                tc3 = spool.tile([52, B], f32, tag="tc3")
                nc.scalar.activation(tc3[32:52, :], c1[:, :], AF.Tanh)
                hst = spool.tile([D, B], fmv, tag="hst")
                nc.vector.tensor_mul(hst[:, :], sio[32:52, :], tc3[32:52, :])
                # 1-input copy may shift partition base; 2-input ops cannot
                nc.vector.tensor_copy(sh[H1 : H1 + D, :], hst[:, :])

                f_ps = ps_f.tile([YD, B], f32)
                nc.tensor.matmul(
                    f_ps[:, :], wf[:, :], hst[:, :], start=True, stop=True
                )
                nc.scalar.activation(
                    f_s[:, xo : xo + B],
                    f_ps[:, :],
                    AF.Identity,
                    bias=bft[:, :],
                )
                if xo + B == 1024:
                    nc.gpsimd.dma_start(
                        ftT[:, x0_pos * 1024 : (x0_pos + 1) * 1024], f_s[:, :]
                    )

                for j in range(4):
                    jcols = slice(j * 128, (j + 1) * 128)
                    ta = ps_t.tile([128, GW - 1], f32, tag="tr")
                    nc.tensor.transpose(
                        ta[:, 0 : GW - 1],
                        sh[0 : GW - 1, jcols],
                        ident[0 : GW - 1, 0 : GW - 1],
                    )
                    ah = ahpool.tile([128, GW], f32, tag="ah")
                    nc.vector.tensor_copy(ah[:, 0 : GW - 1], ta[:, 0 : GW - 1])
                    nc.vector.memset(ah[:, GW - 1 : GW], 1.0)
                    nc.tensor.matmul(
                        accG[:, :],
                        ah[:, :],
                        ah[:, :],
                        start=(t == 0),
                        stop=(t == 4 * NH - 1),
                    )
                    t += 1

        oG = outp.tile([GW, GW], f32)
        nc.vector.tensor_copy(oG[:, :], accG[:, :])
        nc.sync.dma_start(sG[:, :], oG[:, :])

    nc.compile()
    _PROG[key] = nc
    return nc


def _prep_inputs(inputs):
    node_repr = np.ascontiguousarray(np.asarray(inputs["node_repr"], np.float32))
    emb_table = np.ascontiguousarray(np.asarray(inputs["emb_table"], np.float32))
    W1 = np.asarray(inputs["W1"], np.float32)
    b1 = np.asarray(inputs["b1"], np.float32)
    W_ih = np.asarray(inputs["W_ih"], np.float32)
    b_ih = np.asarray(inputs["b_ih"], np.float32)
    b_hh = np.asarray(inputs["b_hh"], np.float32)
    Wf = np.asarray(inputs["Wf"], np.float32)
    bf = np.asarray(inputs["bf"], np.float32)
    idx = np.asarray(inputs["context"])[:, 0].astype(np.int64)

    w1T = np.ascontiguousarray(W1.T)
    b1c = np.ascontiguousarray(b1.reshape(H1, 1))

    # torch gate order i,f,g,o; place [i@0:20 | o@32:52 | g@64:84], padded so
    # every partition slice starts at a multiple of 32. f-gate unused (c0 = 0).
    bsum = b_ih + b_hh
    wgT = np.zeros((EMB, 96), np.float32)
    bg96 = np.zeros((96, 1), np.float32)
    wgT[:, 0:20] = W_ih[0:20].T
    bg96[0:20, 0] = bsum[0:20]
    wgT[:, 32:52] = W_ih[60:80].T
    bg96[32:52, 0] = bsum[60:80]
    wgT[:, 64:84] = W_ih[40:60].T
    bg96[64:84, 0] = bsum[40:60]

    wfT = np.ascontiguousarray(Wf.T)
    bf2 = np.ascontiguousarray(bf.reshape(YD, 1))

    in_maps = []
    for c in range(NCORES):
        rows = slice(c * MC, (c + 1) * MC)
        xTc = np.ascontiguousarray(node_repr[rows].T)
        x0Tc = np.ascontiguousarray(emb_table[idx[rows]].T)
        in_maps.append(
            {
                "xT": xTc,
                "x0T": x0Tc,
                "w1T": w1T,
                "wgT": wgT,
                "b1c": b1c,
                "bg96": bg96,
                "wfT": wfT,
                "bf2": bf2,
            }
        )
    return in_maps


def kernel(**inputs):
    global LAST_EXEC_NS, LAST_RESULTS
    from concourse.bass_utils import run_bass_kernel_spmd

    in_maps = _prep_inputs(inputs)
    nc = _build()
    trace = bool(os.environ.get("KERNEL_TRACE"))
    res = run_bass_kernel_spmd(nc, in_maps, list(range(NCORES)), trace=trace)
    LAST_EXEC_NS = res.exec_time_ns
    LAST_RESULTS = res

    A = np.zeros((H1, H1), np.float64)
    C = np.zeros((H1, D), np.float64)
    suma = np.zeros(H1, np.float64)
    Hh = np.zeros((D, D), np.float64)
    sumh = np.zeros(D, np.float64)
    ft = np.empty((M, YD), np.float32)
    for c, r in enumerate(res.results):
        G = r["sG"].astype(np.float64)
        A += G[0:H1, 0:H1]
        C += G[0:H1, H1 : H1 + D]
        suma += G[0:H1, H1 + D]
        Hh += G[H1 : H1 + D, H1 : H1 + D]
        sumh += G[H1 : H1 + D, H1 + D]
        ft[c * MC : (c + 1) * MC] = r["ftT"].T

    W2 = np.asarray(inputs["W2"], np.float64)
    b2 = np.asarray(inputs["b2"], np.float64)
    m = float(M)
    W2s = W2 @ suma
    sumg = W2s + m * b2
    Ggg = (
        W2 @ A @ W2.T
        + np.outer(W2s, b2)
        + np.outer(b2, W2s)
        + m * np.outer(b2, b2)
    )
    Ggh = W2 @ C + np.outer(b2, sumh)
    mg = sumg / m
    mh = sumh / m
    S11c = (Ggg - m * np.outer(mg, mg)) / (m - 1.0)
    S22c = (Hh - m * np.outer(mh, mh)) / (m - 1.0)
    S12 = (Ggh - m * np.outer(mg, mh)) / (m - 1.0)
    var_g = np.float32(np.mean(np.diag(S11c)))
    var_f = np.float32(np.mean(np.diag(S22c)))

    eye = np.eye(D, dtype=np.float64)

    def inv_sqrtm(S):
        w, V = np.linalg.eigh(S)
        return (V * (1.0 / np.sqrt(w))) @ V.T

    T = inv_sqrtm(S11c + REG1 * eye) @ S12 @ inv_sqrtm(S22c + REG2 * eye)
    s = np.linalg.svd(T, compute_uv=False)
    loss = np.float32(-np.sum(s))
    return (loss, var_g, var_f, ft)


# revision 10
# speedup vs baseline: 1.1032x; 1.1032x over previous
"""Bass/Trainium2 kernel for nn_GraphTextModel (loss_fn).

Strategy (data-parallel over m=65536 rows, 8 cores x 8192 rows):
  Host:   pre-transpose x / gathered-embedding slices so the PE contracts
          along partitions; gather emb_table rows; spread LSTM gate weights
          to 32-aligned partition groups [i@0:20 | o@32:52 | g@64:84]
          (f-gate unused since c0=0).
  Device: aT = W1T.T @ xT, relu+bias via ACT      -> sh[0:64]
          gatesT = WgT.T @ x0T (+bias in ACT), sigmoid/tanh -> h1sT
          ftT = Wf @ h1sT (+bf in ACT)            -> output [64, 8192]
          one PE-transpose per 128-row chunk of the stacked [a|h] tile,
          one Gram matmul into a PSUM accumulator:
             sG = [a|h|1]^T @ [a|h|1]  (85x85, PSUM-resident)
  Host:   combine the 8 partial Gram stats, form the 20x20 sigma matrices,
          inv-sqrtm (eigh) + SVD in float64, assemble (loss, var_g, var_f, ft).

The big matmul movers use float32r (relaxed-precision fp32 multiply at 1
cycle/row vs 4 for full fp32); the Gram/transpose/ft path stays full fp32.
"""

import os
import sys

import numpy as np

try:  # pragma: no cover - environment plumbing
    import concourse  # noqa: F401
except ImportError:
    sys.path.insert(0, "/opt/trn_rl_repo")

M = 65536
PSD = 1024
H1 = 64
D = 20
EMB = 300
YD = 64
NCORES = 8
MC = M // NCORES  # 8192 rows per core
BB = 1024  # columns per DMA block (4 KiB bursts)
NBB = MC // BB  # 8
B = 512  # columns per compute half-block (PSUM bank limit for fp32)
KX = PSD // 128  # 8 contraction chunks for node_repr
GW = 85  # gram width: 64 (a) + 20 (h) + 1 (ones)
REG1 = 0.001
REG2 = 0.001

F32R = os.environ.get("KERNEL_F32R", "1") == "1"

_PROG = {}
LAST_EXEC_NS = None
LAST_RESULTS = None


def _build():
    key = ("nc", F32R)
    if key in _PROG:
        return _PROG[key]
    from contextlib import ExitStack

    import concourse.tile as tile
    from concourse import bacc, mybir
    from concourse.masks import make_identity

    f32 = mybir.dt.float32
    fmv = mybir.dt.float32r if F32R else f32  # dtype of the big movers
    bf16 = mybir.dt.bfloat16
    AF = mybir.ActivationFunctionType

    nc = bacc.Bacc(
        "TRN2", target_bir_lowering=False, debug=False, num_devices=NCORES
    )

    xT = nc.dram_tensor("xT", [PSD, MC], fmv, kind="ExternalInput").ap()
    x0T = nc.dram_tensor("x0T", [320, MC], bf16, kind="ExternalInput").ap()
    w1T = nc.dram_tensor("w1T", [PSD, H1], fmv, kind="ExternalInput").ap()
    wgT = nc.dram_tensor("wgT", [320, 96], bf16, kind="ExternalInput").ap()
    biasp = nc.dram_tensor("biasp", [128, 3], f32, kind="ExternalInput").ap()
    wfT = nc.dram_tensor("wfT", [D, YD], fmv, kind="ExternalInput").ap()

    ftT = nc.dram_tensor("ftT", [YD, MC], f32, kind="ExternalOutput").ap()
    sG = nc.dram_tensor("sG", [GW, GW], f32, kind="ExternalOutput").ap()

    with tile.TileContext(nc) as tc, ExitStack() as ctx:
        wpool = ctx.enter_context(tc.tile_pool(name="w", bufs=1))
        xpool = ctx.enter_context(tc.tile_pool(name="xp", bufs=2))
        x0pool = ctx.enter_context(tc.tile_pool(name="x0p", bufs=3))
        spool = ctx.enter_context(tc.tile_pool(name="sp", bufs=2))
        fpool = ctx.enter_context(tc.tile_pool(name="fp", bufs=2))
        ahpool = ctx.enter_context(tc.tile_pool(name="ahp", bufs=4))
        outp = ctx.enter_context(tc.tile_pool(name="outp", bufs=2))

        ps_a = ctx.enter_context(tc.tile_pool(name="ps_a", bufs=2, space="PSUM"))
        ps_g = ctx.enter_context(tc.tile_pool(name="ps_g", bufs=2, space="PSUM"))
        ps_f = ctx.enter_context(tc.tile_pool(name="ps_f", bufs=1, space="PSUM"))
        ps_t = ctx.enter_context(tc.tile_pool(name="ps_t", bufs=2, space="PSUM"))
        ps_acc = ctx.enter_context(tc.tile_pool(name="ps_acc", bufs=1, space="PSUM"))

        ident = wpool.tile([128, 128], f32)
        make_identity(nc, ident[:, :])

        w1 = wpool.tile([128, KX * H1], fmv)
        for c in range(KX):
            nc.gpsimd.dma_start(
                w1[:, c * H1 : (c + 1) * H1], w1T[c * 128 : (c + 1) * 128, :]
            )
        wg = wpool.tile([128, 288], bf16)
        nc.gpsimd.dma_start(wg[:, 0:96], wgT[0:128, :])
        nc.gpsimd.dma_start(wg[:, 96:192], wgT[128:256, :])
        nc.gpsimd.dma_start(wg[0:64, 192:288], wgT[256:320, :])

        bp = wpool.tile([128, 3], f32)
        nc.gpsimd.dma_start(bp[:, :], biasp[:, :])
        b1t = bp[0:H1, 0:1]
        bgt = bp[0:96, 1:2]
        bft = bp[0:YD, 2:3]
        wf = wpool.tile([D, YD], fmv)
        nc.gpsimd.dma_start(wf[:, :], wfT[:, :])

        accG = ps_acc.tile([GW, GW], f32)

        # variable-width x blocks: 1024 for pipeline ramp-up, then 2048 so
        # DMA packets reach 8 KiB/partition-run; x0/gates stay 1024-granular
        XBLOCKS = [(0, 1024), (1024, 1024), (2048, 2048), (4096, 2048), (6144, 2048)]
        NH = MC // B  # 16 compute halves of 512 cols
        x0t = None
        x0_pos = -1
        t = 0
        for xoff, xw in XBLOCKS:
            xt = xpool.tile([128, KX * xw], fmv, tag="xt")
            for c in range(KX):
                nc.sync.dma_start(
                    xt[:, c * xw : (c + 1) * xw],
                    xT[c * 128 : (c + 1) * 128, xoff : xoff + xw],
                )
            for hh in range(xw // B):
                pos = xoff + hh * B
                if pos // 1024 != x0_pos:
                    x0_pos = pos // 1024
                    x0c = slice(x0_pos * 1024, (x0_pos + 1) * 1024)
                    x0t = x0pool.tile([128, 3 * 1024], bf16, tag="x0t")
                    nc.gpsimd.dma_start(x0t[:, 0:1024], x0T[0:128, x0c])
                    nc.gpsimd.dma_start(x0t[:, 1024:2048], x0T[128:256, x0c])
                    nc.gpsimd.dma_start(x0t[0:64, 2048:3072], x0T[256:320, x0c])
                    f_s = fpool.tile([YD, 1024], f32, tag="f_s")
                xo = pos - x0_pos * 1024

                a_ps = ps_a.tile([H1, B], f32)
                for c in range(KX):
                    nc.tensor.matmul(
                        a_ps[:, :],
                        w1[:, c * H1 : (c + 1) * H1],
                        xt[:, c * xw + hh * B : c * xw + hh * B + B],
                        start=(c == 0),
                        stop=(c == KX - 1),
                    )

                g_ps = ps_g.tile([96, B], f32)
                nc.tensor.matmul(
                    g_ps[:, :],
                    wg[:, 0:96],
                    x0t[:, xo : xo + B],
                    start=True,
                    stop=False,
                )
                nc.tensor.matmul(
                    g_ps[:, :],
                    wg[:, 96:192],
                    x0t[:, 1024 + xo : 1024 + xo + B],
                    start=False,
                    stop=False,
                )
                nc.tensor.matmul(
                    g_ps[:, :],
                    wg[0:64, 192:288],
                    x0t[0:64, 2048 + xo : 2048 + xo + B],
                    start=False,
                    stop=True,
                )

                # stacked [a | h] tile: relu(a)+b1 -> rows 0:64, h1s -> 64:84
                sh = spool.tile([H1 + D, B], f32, tag="sh")
                nc.scalar.activation(
                    sh[0:H1, :], a_ps[:, :], AF.Relu, bias=b1t
                )

                # one sigmoid over [i@0:20 | junk | o@32:52]
                sio = spool.tile([52, B], f32, tag="sio")
                nc.scalar.activation(
                    sio[:, :], g_ps[0:52, :], AF.Sigmoid, bias=bgt[0:52, 0:1]
                )
                tg = spool.tile([D, B], f32, tag="tg")
                nc.scalar.activation(
                    tg[:, :], g_ps[64:84, :], AF.Tanh, bias=bgt[64:84, 0:1]
                )
                c1 = spool.tile([D, B], f32, tag="c1")
                nc.vector.tensor_mul(c1[:, :], sio[0:20, :], tg[:, :])
                # tanh(c1) written at base 32 so the h-mul has equal-base ins
                tc3 = spool.tile([52, B], f32, tag="tc3")
                nc.scalar.activation(tc3[32:52, :], c1[:, :], AF.Tanh)
                hst = spool.tile([D, B], fmv, tag="hst")
                nc.vector.tensor_mul(hst[:, :], sio[32:52, :], tc3[32:52, :])
                # 1-input copy may shift partition base; 2-input ops cannot
                nc.vector.tensor_copy(sh[H1 : H1 + D, :], hst[:, :])

                f_ps = ps_f.tile([YD, B], f32)
                nc.tensor.matmul(
                    f_ps[:, :], wf[:, :], hst[:, :], start=True, stop=True
                )
                nc.scalar.activation(
                    f_s[:, xo : xo + B],
                    f_ps[:, :],
                    AF.Identity,
                    bias=bft,
                )
                if xo + B == 1024:
                    nc.gpsimd.dma_start(
                        ftT[:, x0_pos * 1024 : (x0_pos + 1) * 1024], f_s[:, :]
                    )

                for j in range(4):
                    jcols = slice(j * 128, (j + 1) * 128)
                    ta = ps_t.tile([128, GW - 1], f32, tag="tr")
                    nc.tensor.transpose(
                        ta[:, 0 : GW - 1],
                        sh[0 : GW - 1, jcols],
                        ident[0 : GW - 1, 0 : GW - 1],
                    )
                    ah = ahpool.tile([128, GW], f32, tag="ah")
                    nc.vector.tensor_copy(ah[:, 0 : GW - 1], ta[:, 0 : GW - 1])
                    nc.vector.memset(ah[:, GW - 1 : GW], 1.0)
                    nc.tensor.matmul(
                        accG[:, :],
                        ah[:, :],
                        ah[:, :],
                        start=(t == 0),
                        stop=(t == 4 * NH - 1),
                    )
                    t += 1

        oG = outp.tile([GW, GW], f32)
        nc.vector.tensor_copy(oG[:, :], accG[:, :])
        nc.sync.dma_start(sG[:, :], oG[:, :])

    nc.compile()
    _PROG[key] = nc
    return nc


def _prep_inputs(inputs):
    node_repr = np.ascontiguousarray(np.asarray(inputs["node_repr"], np.float32))
    emb_table = np.ascontiguousarray(np.asarray(inputs["emb_table"], np.float32))
    W1 = np.asarray(inputs["W1"], np.float32)
    b1 = np.asarray(inputs["b1"], np.float32)
    W_ih = np.asarray(inputs["W_ih"], np.float32)
    b_ih = np.asarray(inputs["b_ih"], np.float32)
    b_hh = np.asarray(inputs["b_hh"], np.float32)
    Wf = np.asarray(inputs["Wf"], np.float32)
    bf = np.asarray(inputs["bf"], np.float32)
    idx = np.asarray(inputs["context"])[:, 0].astype(np.int64)

    w1T = np.ascontiguousarray(W1.T)

    import ml_dtypes

    # torch gate order i,f,g,o; place [i@0:20 | o@32:52 | g@64:84], padded so
    # every partition slice starts at a multiple of 32. f-gate unused (c0 = 0).
    bsum = b_ih + b_hh
    wgT = np.zeros((320, 96), np.float32)
    bg96 = np.zeros((96, 1), np.float32)
    wgT[0:EMB, 0:20] = W_ih[0:20].T
    bg96[0:20, 0] = bsum[0:20]
    wgT[0:EMB, 32:52] = W_ih[60:80].T
    bg96[32:52, 0] = bsum[60:80]
    wgT[0:EMB, 64:84] = W_ih[40:60].T
    bg96[64:84, 0] = bsum[40:60]
    wgT = wgT.astype(ml_dtypes.bfloat16)

    wfT = np.ascontiguousarray(Wf.T)

    biasp = np.zeros((128, 3), np.float32)
    biasp[0:H1, 0] = b1
    biasp[0:96, 1] = bg96[:, 0]
    biasp[0:YD, 2] = bf

    in_maps = []
    for c in range(NCORES):
        rows = slice(c * MC, (c + 1) * MC)
        xTc = np.ascontiguousarray(node_repr[rows].T)
        x0Tc = np.zeros((320, MC), ml_dtypes.bfloat16)
        x0Tc[0:EMB] = emb_table[idx[rows]].T.astype(ml_dtypes.bfloat16)
        in_maps.append(
            {
                "xT": xTc,
                "x0T": x0Tc,
                "w1T": w1T,
                "wgT": wgT,
                "biasp": biasp,
                "wfT": wfT,
            }
        )
    return in_maps


def kernel(**inputs):
    global LAST_EXEC_NS, LAST_RESULTS
    from concourse.bass_utils import run_bass_kernel_spmd

    in_maps = _prep_inputs(inputs)
    nc = _build()
    trace = bool(os.environ.get("KERNEL_TRACE"))
    res = run_bass_kernel_spmd(nc, in_maps, list(range(NCORES)), trace=trace)
    LAST_EXEC_NS = res.exec_time_ns
    LAST_RESULTS = res

    A = np.zeros((H1, H1), np.float64)
    C = np.zeros((H1, D), np.float64)
    suma = np.zeros(H1, np.float64)
    Hh = np.zeros((D, D), np.float64)
    sumh = np.zeros(D, np.float64)
    ft = np.empty((M, YD), np.float32)
    for c, r in enumerate(res.results):
        G = r["sG"].astype(np.float64)
        A += G[0:H1, 0:H1]
        C += G[0:H1, H1 : H1 + D]
        suma += G[0:H1, H1 + D]
        Hh += G[H1 : H1 + D, H1 : H1 + D]
        sumh += G[H1 : H1 + D, H1 + D]
        ft[c * MC : (c + 1) * MC] = r["ftT"].T

    W2 = np.asarray(inputs["W2"], np.float64)
    b2 = np.asarray(inputs["b2"], np.float64)
    m = float(M)
    W2s = W2 @ suma
    sumg = W2s + m * b2
    Ggg = (
        W2 @ A @ W2.T
        + np.outer(W2s, b2)
        + np.outer(b2, W2s)
        + m * np.outer(b2, b2)
    )
    Ggh = W2 @ C + np.outer(b2, sumh)
    mg = sumg / m
    mh = sumh / m
    S11c = (Ggg - m * np.outer(mg, mg)) / (m - 1.0)
    S22c = (Hh - m * np.outer(mh, mh)) / (m - 1.0)
    S12 = (Ggh - m * np.outer(mg, mh)) / (m - 1.0)
    var_g = np.float32(np.mean(np.diag(S11c)))
    var_f = np.float32(np.mean(np.diag(S22c)))

    eye = np.eye(D, dtype=np.float64)

    def inv_sqrtm(S):
        w, V = np.linalg.eigh(S)
        return (V * (1.0 / np.sqrt(w))) @ V.T

    T = inv_sqrtm(S11c + REG1 * eye) @ S12 @ inv_sqrtm(S22c + REG2 * eye)
    s = np.linalg.svd(T, compute_uv=False)
    loss = np.float32(-np.sum(s))
    return (loss, var_g, var_f, ft)


# revision 11
# speedup vs baseline: 1.1967x; 1.0848x over previous
"""Bass/Trainium2 kernel for nn_GraphTextModel (loss_fn).

Strategy (data-parallel over m=65536 rows, 8 cores x 8192 rows):
  Host:   pre-transpose x / gathered-embedding slices so the PE contracts
          along partitions; gather emb_table rows; spread LSTM gate weights
          to 32-aligned partition groups [i@0:20 | o@32:52 | g@64:84]
          (f-gate unused since c0=0).
  Device: aT = W1T.T @ xT, relu+bias via ACT      -> sh[0:64]
          gatesT = WgT.T @ x0T (+bias in ACT), sigmoid/tanh -> h1sT
          ftT = Wf @ h1sT (+bf in ACT)            -> output [64, 8192]
          one PE-transpose per 128-row chunk of the stacked [a|h] tile,
          one Gram matmul into a PSUM accumulator:
             sG = [a|h|1]^T @ [a|h|1]  (85x85, PSUM-resident)
  Host:   combine the 8 partial Gram stats, form the 20x20 sigma matrices,
          inv-sqrtm (eigh) + SVD in float64, assemble (loss, var_g, var_f, ft).

The big matmul movers use float32r (relaxed-precision fp32 multiply at 1
cycle/row vs 4 for full fp32); the Gram/transpose/ft path stays full fp32.
"""

import os
import sys

import numpy as np

try:  # pragma: no cover - environment plumbing
    import concourse  # noqa: F401
except ImportError:
    sys.path.insert(0, "/opt/trn_rl_repo")

M = 65536
PSD = 1024
H1 = 64
D = 20
EMB = 300
YD = 64
NCORES = 8
MC = M // NCORES  # 8192 rows per core
BB = 1024  # columns per DMA block (4 KiB bursts)
NBB = MC // BB  # 8
B = 512  # columns per compute half-block (PSUM bank limit for fp32)
KX = PSD // 128  # 8 contraction chunks for node_repr
GW = 85  # gram width: 64 (a) + 20 (h) + 1 (ones)
REG1 = 0.001
REG2 = 0.001

F32R = os.environ.get("KERNEL_F32R", "1") == "1"

_PROG = {}
LAST_EXEC_NS = None
LAST_RESULTS = None


def _build():
    key = ("nc", F32R)
    if key in _PROG:
        return _PROG[key]
    from contextlib import ExitStack

    import concourse.tile as tile
    from concourse import bacc, mybir
    from concourse.masks import make_identity

    f32 = mybir.dt.float32
    fmv = mybir.dt.float32r if F32R else f32  # dtype of the big movers
    bf16 = mybir.dt.bfloat16
    AF = mybir.ActivationFunctionType

    nc = bacc.Bacc(
        "TRN2", target_bir_lowering=False, debug=False, num_devices=NCORES
    )

    xT = nc.dram_tensor("xT", [PSD, MC], fmv, kind="ExternalInput").ap()
    x0T = nc.dram_tensor("x0T", [320, MC], bf16, kind="ExternalInput").ap()
    w1T = nc.dram_tensor("w1T", [PSD, H1], fmv, kind="ExternalInput").ap()
    wgT = nc.dram_tensor("wgT", [320, 96], bf16, kind="ExternalInput").ap()
    biasp = nc.dram_tensor("biasp", [128, 3], f32, kind="ExternalInput").ap()
    wfT = nc.dram_tensor("wfT", [D, YD], fmv, kind="ExternalInput").ap()

    ftT = nc.dram_tensor("ftT", [YD, MC], f32, kind="ExternalOutput").ap()
    sG = nc.dram_tensor("sG", [GW, GW], f32, kind="ExternalOutput").ap()

    with tile.TileContext(nc) as tc, ExitStack() as ctx:
        wpool = ctx.enter_context(tc.tile_pool(name="w", bufs=1))
        xpool = ctx.enter_context(tc.tile_pool(name="xp", bufs=2))
        x0pool = ctx.enter_context(tc.tile_pool(name="x0p", bufs=3))
        spool = ctx.enter_context(tc.tile_pool(name="sp", bufs=2))
        fpool = ctx.enter_context(tc.tile_pool(name="fp", bufs=2))
        ahpool = ctx.enter_context(tc.tile_pool(name="ahp", bufs=4))
        outp = ctx.enter_context(tc.tile_pool(name="outp", bufs=2))

        ps_a = ctx.enter_context(tc.tile_pool(name="ps_a", bufs=2, space="PSUM"))
        ps_g = ctx.enter_context(tc.tile_pool(name="ps_g", bufs=2, space="PSUM"))
        ps_f = ctx.enter_context(tc.tile_pool(name="ps_f", bufs=1, space="PSUM"))
        ps_t = ctx.enter_context(tc.tile_pool(name="ps_t", bufs=2, space="PSUM"))
        ps_acc = ctx.enter_context(tc.tile_pool(name="ps_acc", bufs=1, space="PSUM"))

        ident = wpool.tile([128, 128], f32)
        make_identity(nc, ident[:, :])

        w1 = wpool.tile([128, KX * H1], fmv)
        for c in range(KX):
            nc.gpsimd.dma_start(
                w1[:, c * H1 : (c + 1) * H1], w1T[c * 128 : (c + 1) * 128, :]
            )
        wg = wpool.tile([128, 288], bf16)
        nc.gpsimd.dma_start(wg[:, 0:96], wgT[0:128, :])
        nc.gpsimd.dma_start(wg[:, 96:192], wgT[128:256, :])
        nc.gpsimd.dma_start(wg[0:64, 192:288], wgT[256:320, :])

        bp = wpool.tile([128, 3], f32)
        nc.gpsimd.dma_start(bp[:, :], biasp[:, :])
        b1t = bp[0:H1, 0:1]
        bgt = bp[0:96, 1:2]
        bft = bp[0:YD, 2:3]
        wf = wpool.tile([D, YD], fmv)
        nc.gpsimd.dma_start(wf[:, :], wfT[:, :])

        accG = ps_acc.tile([GW, GW], f32)

        # variable-width x blocks: 1024 for pipeline ramp-up, then 2048 so
        # DMA packets reach 8 KiB/partition-run; x0/gates stay 1024-granular
        XBLOCKS = [(0, 1024), (1024, 1024), (2048, 2048), (4096, 2048), (6144, 1024), (7168, 1024)]
        NH = MC // B  # 16 compute halves of 512 cols
        x0t = None
        x0_pos = -1
        t = 0
        for xoff, xw in XBLOCKS:
            xt = xpool.tile([128, KX * xw], fmv, tag="xt")
            for c in range(KX):
                nc.sync.dma_start(
                    xt[:, c * xw : (c + 1) * xw],
                    xT[c * 128 : (c + 1) * 128, xoff : xoff + xw],
                )
            for hh in range(xw // B):
                pos = xoff + hh * B
                if pos // 1024 != x0_pos:
                    x0_pos = pos // 1024
                    x0c = slice(x0_pos * 1024, (x0_pos + 1) * 1024)
                    x0t = x0pool.tile([128, 3 * 1024], bf16, tag="x0t")
                    nc.gpsimd.dma_start(x0t[:, 0:1024], x0T[0:128, x0c])
                    nc.gpsimd.dma_start(x0t[:, 1024:2048], x0T[128:256, x0c])
                    nc.gpsimd.dma_start(x0t[0:64, 2048:3072], x0T[256:320, x0c])
                    f_s = fpool.tile([YD, 1024], f32, tag="f_s")
                xo = pos - x0_pos * 1024

                a_ps = ps_a.tile([H1, B], f32)
                for c in range(KX):
                    nc.tensor.matmul(
                        a_ps[:, :],
                        w1[:, c * H1 : (c + 1) * H1],
                        xt[:, c * xw + hh * B : c * xw + hh * B + B],
                        start=(c == 0),
                        stop=(c == KX - 1),
                    )

                g_ps = ps_g.tile([96, B], f32)
                nc.tensor.matmul(
                    g_ps[:, :],
                    wg[:, 0:96],
                    x0t[:, xo : xo + B],
                    start=True,
                    stop=False,
                )
                nc.tensor.matmul(
                    g_ps[:, :],
                    wg[:, 96:192],
                    x0t[:, 1024 + xo : 1024 + xo + B],
                    start=False,
                    stop=False,
                )
                nc.tensor.matmul(
                    g_ps[:, :],
                    wg[0:64, 192:288],
                    x0t[0:64, 2048 + xo : 2048 + xo + B],
                    start=False,
                    stop=True,
                )

                # stacked [a | h] tile: relu(a)+b1 -> rows 0:64, h1s -> 64:84
                sh = spool.tile([H1 + D, B], f32, tag="sh")
                nc.scalar.activation(
                    sh[0:H1, :], a_ps[:, :], AF.Relu, bias=b1t
                )

                # one sigmoid over [i@0:20 | junk | o@32:52]
                sio = spool.tile([52, B], f32, tag="sio")
                nc.scalar.activation(
                    sio[:, :], g_ps[0:52, :], AF.Sigmoid, bias=bgt[0:52, 0:1]
                )
                tg = spool.tile([D, B], f32, tag="tg")
                nc.scalar.activation(
                    tg[:, :], g_ps[64:84, :], AF.Tanh, bias=bgt[64:84, 0:1]
                )
                c1 = spool.tile([D, B], f32, tag="c1")
                nc.vector.tensor_mul(c1[:, :], sio[0:20, :], tg[:, :])
                # tanh(c1) written at base 32 so the h-mul has equal-base ins
                tc3 = spool.tile([52, B], f32, tag="tc3")
                nc.scalar.activation(tc3[32:52, :], c1[:, :], AF.Tanh)
                hst = spool.tile([D, B], fmv, tag="hst")
                nc.vector.tensor_mul(hst[:, :], sio[32:52, :], tc3[32:52, :])
                # 1-input copy may shift partition base; 2-input ops cannot
                nc.vector.tensor_copy(sh[H1 : H1 + D, :], hst[:, :])

                f_ps = ps_f.tile([YD, B], f32)
                nc.tensor.matmul(
                    f_ps[:, :], wf[:, :], hst[:, :], start=True, stop=True
                )
                nc.scalar.activation(
                    f_s[:, xo : xo + B],
                    f_ps[:, :],
                    AF.Identity,
                    bias=bft,
                )
                if xo + B == 1024:
                    nc.gpsimd.dma_start(
                        ftT[:, x0_pos * 1024 : (x0_pos + 1) * 1024], f_s[:, :]
                    )

                for j in range(4):
                    jcols = slice(j * 128, (j + 1) * 128)
                    ta = ps_t.tile([128, GW - 1], f32, tag="tr")
                    nc.tensor.transpose(
                        ta[:, 0 : GW - 1],
                        sh[0 : GW - 1, jcols],
                        ident[0 : GW - 1, 0 : GW - 1],
                    )
                    ah = ahpool.tile([128, GW], bf16, tag="ah")
                    nc.vector.tensor_copy(ah[:, 0 : GW - 1], ta[:, 0 : GW - 1])
                    nc.vector.memset(ah[:, GW - 1 : GW], 1.0)
                    nc.tensor.matmul(
                        accG[:, :],
                        ah[:, :],
                        ah[:, :],
                        start=(t == 0),
                        stop=(t == 4 * NH - 1),
                    )
                    t += 1

        oG = outp.tile([GW, GW], f32)
        nc.vector.tensor_copy(oG[:, :], accG[:, :])
        nc.sync.dma_start(sG[:, :], oG[:, :])

    nc.compile()
    _PROG[key] = nc
    return nc


def _prep_inputs(inputs):
    node_repr = np.ascontiguousarray(np.asarray(inputs["node_repr"], np.float32))
    emb_table = np.ascontiguousarray(np.asarray(inputs["emb_table"], np.float32))
    W1 = np.asarray(inputs["W1"], np.float32)
    b1 = np.asarray(inputs["b1"], np.float32)
    W_ih = np.asarray(inputs["W_ih"], np.float32)
    b_ih = np.asarray(inputs["b_ih"], np.float32)
    b_hh = np.asarray(inputs["b_hh"], np.float32)
    Wf = np.asarray(inputs["Wf"], np.float32)
    bf = np.asarray(inputs["bf"], np.float32)
    idx = np.asarray(inputs["context"])[:, 0].astype(np.int64)

    w1T = np.ascontiguousarray(W1.T)

    import ml_dtypes

    # torch gate order i,f,g,o; place [i@0:20 | o@32:52 | g@64:84], padded so
    # every partition slice starts at a multiple of 32. f-gate unused (c0 = 0).
    bsum = b_ih + b_hh
    wgT = np.zeros((320, 96), np.float32)
    bg96 = np.zeros((96, 1), np.float32)
    wgT[0:EMB, 0:20] = W_ih[0:20].T
    bg96[0:20, 0] = bsum[0:20]
    wgT[0:EMB, 32:52] = W_ih[60:80].T
    bg96[32:52, 0] = bsum[60:80]
    wgT[0:EMB, 64:84] = W_ih[40:60].T
    bg96[64:84, 0] = bsum[40:60]
    wgT = wgT.astype(ml_dtypes.bfloat16)

    wfT = np.ascontiguousarray(Wf.T)

    biasp = np.zeros((128, 3), np.float32)
    biasp[0:H1, 0] = b1
    biasp[0:96, 1] = bg96[:, 0]
    biasp[0:YD, 2] = bf

    in_maps = []
    for c in range(NCORES):
        rows = slice(c * MC, (c + 1) * MC)
        xTc = np.ascontiguousarray(node_repr[rows].T)
        x0Tc = np.zeros((320, MC), ml_dtypes.bfloat16)
        x0Tc[0:EMB] = emb_table[idx[rows]].T.astype(ml_dtypes.bfloat16)
        in_maps.append(
            {
                "xT": xTc,
                "x0T": x0Tc,
                "w1T": w1T,
                "wgT": wgT,
                "biasp": biasp,
                "wfT": wfT,
            }
        )
    return in_maps


def kernel(**inputs):
    global LAST_EXEC_NS, LAST_RESULTS
    from concourse.bass_utils import run_bass_kernel_spmd

    in_maps = _prep_inputs(inputs)
    nc = _build()
    trace = bool(os.environ.get("KERNEL_TRACE"))
    res = run_bass_kernel_spmd(nc, in_maps, list(range(NCORES)), trace=trace)
    LAST_EXEC_NS = res.exec_time_ns
    LAST_RESULTS = res

    A = np.zeros((H1, H1), np.float64)
    C = np.zeros((H1, D), np.float64)
    suma = np.zeros(H1, np.float64)
    Hh = np.zeros((D, D), np.float64)
    sumh = np.zeros(D, np.float64)
    ft = np.empty((M, YD), np.float32)
    for c, r in enumerate(res.results):
        G = r["sG"].astype(np.float64)
        A += G[0:H1, 0:H1]
        C += G[0:H1, H1 : H1 + D]
        suma += G[0:H1, H1 + D]
        Hh += G[H1 : H1 + D, H1 : H1 + D]
        sumh += G[H1 : H1 + D, H1 + D]
        ft[c * MC : (c + 1) * MC] = r["ftT"].T

    W2 = np.asarray(inputs["W2"], np.float64)
    b2 = np.asarray(inputs["b2"], np.float64)
    m = float(M)
    W2s = W2 @ suma
    sumg = W2s + m * b2
    Ggg = (
        W2 @ A @ W2.T
        + np.outer(W2s, b2)
        + np.outer(b2, W2s)
        + m * np.outer(b2, b2)
    )
    Ggh = W2 @ C + np.outer(b2, sumh)
    mg = sumg / m
    mh = sumh / m
    S11c = (Ggg - m * np.outer(mg, mg)) / (m - 1.0)
    S22c = (Hh - m * np.outer(mh, mh)) / (m - 1.0)
    S12 = (Ggh - m * np.outer(mg, mh)) / (m - 1.0)
    var_g = np.float32(np.mean(np.diag(S11c)))
    var_f = np.float32(np.mean(np.diag(S22c)))

    eye = np.eye(D, dtype=np.float64)

    def inv_sqrtm(S):
        w, V = np.linalg.eigh(S)
        return (V * (1.0 / np.sqrt(w))) @ V.T

    T = inv_sqrtm(S11c + REG1 * eye) @ S12 @ inv_sqrtm(S22c + REG2 * eye)
    s = np.linalg.svd(T, compute_uv=False)
    loss = np.float32(-np.sum(s))
    return (loss, var_g, var_f, ft)
